# revision 12
# baseline (speedup 1.0000x reference)
"""Trainium2 Bass kernel for ragged-sequence attention (fp8-e3m4 stream).

Per batch b:
    tq     = tanh(query[b] @ W + bias)                      [CA, H]
    scores = key[b] @ tq.T                                  [S, CA]
    alpha  = exp(scores) * (s < seq_len[b])                 [S, CA]
    out[b] = (alpha.T @ value[b]) / alpha.sum(axis=0)[:,None]

Strategy (DMA-byte bound; the cost model treats all 16 DMA engines as one
exclusive 360 GB/s resource, so exec time ~ total bytes streamed):
  - Raggedness: independent sub-chunks of each valid prefix (<=128 rows);
    numerator/denominator are additive over s.
  - key/value stream as fp8 e3m4 (4 mantissa bits) for batches with
    L > 256; the error of the normalized attention average shrinks like
    1/sqrt(L_eff), so only short batches need f16. key is pre-scaled x32
    and value x2 to sit in e3m4's narrow range; the exp activation
    applies scale=1/32, bias=-2 (num/den ratio is bias-invariant), and
    the host divides the numerator's x2 back out.
  - Same-batch sub pairs share one tq copy and psum-accumulate into one
    output block (tq + out bytes halved). Leftover singles go in 96-row
    slots (their valid tails are short). Three per-core sections with an
    identical NEFF on every core: e3-pairs[128], e3-singles[96],
    f16-singles[96].
  - No mask / no transpose: invalid rows have zeroed value+ones-col so
    they contribute nothing; scores^T comes straight off the PE with s
    on partitions (kt stationary), and the value matmul runs transposed
    (value chunks stationary) -> out^T w-chunks [128, CA] + den [CA,1].
  - All tq's in one upfront DMA; outputs in f16, DMA'd two groups at a
    time so descriptors are >=512B (below that the model halves DMA
    bandwidth).
"""

import os
import sys

import numpy as np

for _p in ("/opt/trn_rl_repo", "/root/.axon_site/_ro/trn_rl_repo"):
    if os.path.isdir(_p) and _p not in sys.path:
        sys.path.append(_p)

N_CORES = 8
H = 768
HSUB = H // 128  # 6
CA = 32
VW = H + 1       # value cols + ones col
TQW = HSUB * CA  # 192
NQ = HSUB * CA + 1  # out cols per group: 6 w-chunks x 32 + den col

KEY_SCALE = 32.0   # key pre-scaled on host; exp() applies 1/32
VAL_SCALE = 2.0    # value pre-scaled into e3m4 range; host divides out
F16_MAX_L = 256    # batches with L <= this stream in f16 (short batches
                   # have too few terms for fp8 noise to average out)
RSINGLE = 96       # row granularity of single-sub slots

MODE = os.environ.get("BASS_ATTN_MODE", "auto")  # auto | f16

_module_cache = {}


def _subw(rows):
    return HSUB * rows + VW


def _build_module(plan):
    """plan: tuple of (count, nsub, rows, is_f16) sections."""
    import concourse.mybir as mybir
    import concourse.tile as tile
    from concourse import bacc

    f32 = mybir.dt.float32
    f16 = mybir.dt.float16
    e3 = f16 if os.environ.get("BASS_E3_AS_F16") == "1" else mybir.dt.float8e3
    AF = mybir.ActivationFunctionType

    ngrp = sum(c for c, _, _, _ in plan)
    npair = (ngrp + 1) // 2

    nc = bacc.Bacc(None, target_bir_lowering=False, enable_asserts=False)
    tq_d = nc.dram_tensor("tqin", [128, ngrp * TQW], f16, kind="ExternalInput")
    kv_ds = []
    for si, (count, nsub, rows, is_f16) in enumerate(plan):
        kv_ds.append(
            nc.dram_tensor(
                f"kv{si}",
                [count, 128, nsub * _subw(rows)],
                f16 if is_f16 else e3,
                kind="ExternalInput",
            )
        )
    out_d = nc.dram_tensor("outp", [npair, 128, 2, NQ], f16, kind="ExternalOutput")

    with tile.TileContext(nc) as tc:
        with (
            tc.tile_pool(name="tqp", bufs=1) as tqp,
            tc.tile_pool(name="kvp", bufs=ngrp) as kvp,
            tc.tile_pool(name="alp", bufs=3) as alp,
            tc.tile_pool(name="obp", bufs=2) as obp,
            tc.tile_pool(name="ps_s", bufs=2, space="PSUM") as ps_s_pool,
            tc.tile_pool(name="ps_o", bufs=2, space="PSUM") as ps_o_pool,
            tc.tile_pool(name="ps_d", bufs=2, space="PSUM") as ps_d_pool,
        ):
            tq_t = tqp.tile([128, ngrp * TQW], f16, tag="tq")
            nc.sync.dma_start(out=tq_t, in_=tq_d[:, :])

            ob = None
            g = 0
            for si, (count, nsub, rows, is_f16) in enumerate(plan):
                kvdt = f16 if is_f16 else e3
                sw = _subw(rows)
                for j in range(count):
                    kv_t = kvp.tile([128, nsub * sw], kvdt, tag=f"kv{si}")
                    nc.sync.dma_start(out=kv_t, in_=kv_ds[si][j])

                    # scores^T [rows(s), CA] per sub, contracted over h in
                    # 6 chunks of 128; kt stationary puts s on partitions.
                    ps_s = ps_s_pool.tile([128, nsub * CA], f32, tag="ps_s")
                    for m in range(nsub):
                        for ho in range(HSUB):
                            nc.tensor.matmul(
                                ps_s[:rows, m * CA : (m + 1) * CA],
                                lhsT=kv_t[
                                    :, m * sw + ho * rows : m * sw + (ho + 1) * rows
                                ],
                                rhs=tq_t[
                                    :, g * TQW + ho * CA : g * TQW + (ho + 1) * CA
                                ],
                                start=(ho == 0),
                                stop=(ho == HSUB - 1),
                            )

                    al_t = alp.tile([128, nsub * CA], f16, tag="al")
                    nc.scalar.activation(
                        out=al_t[:rows],
                        in_=ps_s[:rows],
                        func=AF.Exp,
                        scale=1.0 / KEY_SCALE,
                    )

                    # out^T accumulated over the group's subs: 6 w-chunks
                    # of [128(w), CA] (value chunk stationary) + den [CA,1].
                    ps_o = ps_o_pool.tile([128, HSUB * CA], f32, tag="ps_o")
                    ps_d = ps_d_pool.tile([CA, 1], f32, tag="ps_d")
                    for q in range(HSUB):
                        for m in range(nsub):
                            voff = m * sw + HSUB * rows
                            nc.tensor.matmul(
                                ps_o[:, q * CA : (q + 1) * CA],
                                lhsT=kv_t[:rows, voff + q * 128 : voff + (q + 1) * 128],
                                rhs=al_t[:rows, m * CA : (m + 1) * CA],
                                start=(m == 0),
                                stop=(m == nsub - 1),
                            )
                    for m in range(nsub):
                        voff = m * sw + HSUB * rows
                        nc.tensor.matmul(
                            ps_d,
                            lhsT=al_t[:rows, m * CA : (m + 1) * CA],
                            rhs=kv_t[:rows, voff + H : voff + H + 1],
                            start=(m == 0),
                            stop=(m == nsub - 1),
                        )

                    if g % 2 == 0:
                        ob = obp.tile([128, 2, NQ], f16, tag="ob")
                    sl_ = g % 2
                    nc.vector.tensor_copy(out=ob[:, sl_, : HSUB * CA], in_=ps_o)
                    nc.vector.tensor_copy(out=ob[:CA, sl_, HSUB * CA :], in_=ps_d)
                    if sl_ == 1:
                        nc.scalar.dma_start(out=out_d[g // 2], in_=ob)
                    elif g == ngrp - 1:
                        nc.scalar.dma_start(
                            out=out_d[g // 2, :, :1], in_=ob[:, :1]
                        )
                    g += 1

    nc.compile()
    return nc


def kernel(key, value, query, seq_len, W, b):
    import ml_dtypes

    key = np.ascontiguousarray(np.asarray(key, dtype=np.float32))
    value = np.ascontiguousarray(np.asarray(value, dtype=np.float32))
    query = np.asarray(query, dtype=np.float32)
    W = np.asarray(W, dtype=np.float32)
    bias = np.asarray(b, dtype=np.float32)
    sl = np.asarray(seq_len).astype(np.int64)

    B, S, H_ = key.shape
    assert H_ == H
    e3np = (
        np.float16
        if os.environ.get("BASS_E3_AS_F16") == "1"
        else ml_dtypes.float8_e3m4
    )

    # host: tiny projection  tq[b] = tanh(query[b] @ W + bias)  [B, CA, H]
    tq = np.tanh(query.reshape(B * query.shape[1], -1) @ W + bias)
    tq = tq.reshape(B, query.shape[1], H).astype(np.float32)
    tq_pack = {
        bi: np.ascontiguousarray(
            tq[bi].T.reshape(HSUB, 128, CA).transpose(1, 0, 2).reshape(128, TQW)
        ).astype(np.float16)
        for bi in range(B)
    }

    # work items per precision class
    pairs = []      # (bi, s0_a, nv_a, s0_b, nv_b)  e3, 128-row subs
    singles8 = []   # (bi, s0, nval)  e3 leftovers, nval <= RSINGLE ideally
    singles16 = []  # (bi, s0, nval)  f16 short batches in 96-row chunks
    for bi in range(B):
        L = int(max(1, min(sl[bi], S)))
        if MODE == "f16" or L <= F16_MAX_L:
            for s0 in range(0, L, RSINGLE):
                singles16.append((bi, s0, min(RSINGLE, L - s0)))
            continue
        subs = [(s0, min(128, L - s0)) for s0 in range(0, L, 128)]
        for i in range(0, len(subs) - 1, 2):
            pairs.append((bi, *subs[i], *subs[i + 1]))
        if len(subs) % 2:
            s0, nv = subs[-1]
            if nv <= RSINGLE:
                singles8.append((bi, s0, nv))
            else:  # split the tail into two <=96-row singles
                h1 = (nv + 1) // 2
                singles8.append((bi, s0, h1))
                singles8.append((bi, s0 + h1, nv - h1))

    nA = -(-len(pairs) // N_CORES) if pairs else 0
    nB8 = -(-len(singles8) // N_CORES) if singles8 else 0
    nB16 = -(-len(singles16) // N_CORES) if singles16 else 0
    plan = tuple(
        s
        for s in (
            (nA, 2, 128, False),
            (nB8, 1, RSINGLE, False),
            (nB16, 1, RSINGLE, True),
        )
        if s[0]
    )
    ngrp = sum(c for c, _, _, _ in plan)
    npair_o = (ngrp + 1) // 2

    tq_in = np.zeros((N_CORES, 128, ngrp * TQW), np.float16)
    kv_arrs = {}
    sec_of = {}  # section index by (nsub, rows, is_f16) signature
    for si, (count, nsub, rows, is_f16) in enumerate(plan):
        dt = np.float16 if is_f16 else e3np
        kv_arrs[si] = np.zeros((N_CORES, count, 128, nsub * _subw(rows)), dt)
        sec_of[(nsub, is_f16)] = si
    slot_map = [[] for _ in range(N_CORES)]  # (global_group, bi)

    def fill_sub(arr, rows, bi, s0, nval):
        # arr: [128, _subw(rows)] view. kt cols [ho*rows + s] x KEY_SCALE;
        # vl rows [s, w] x VAL_SCALE + ones col; invalid rows stay zero.
        kc = key[bi, s0 : s0 + nval] * KEY_SCALE        # [nval, H]
        kt = kc.T.reshape(HSUB, 128, nval).transpose(1, 0, 2)
        ktp = np.zeros((128, HSUB, rows), np.float32)
        ktp[:, :, :nval] = kt
        arr[:, : HSUB * rows] = ktp.reshape(128, HSUB * rows).astype(arr.dtype)
        vb = HSUB * rows
        arr[:nval, vb : vb + H] = (value[bi, s0 : s0 + nval] * VAL_SCALE).astype(
            arr.dtype
        )
        arr[:nval, vb + H] = np.float32(1.0)

    g_base = {}
    acc = 0
    for si, (count, _, _, _) in enumerate(plan):
        g_base[si] = acc
        acc += count

    def place(items, si, nsub, rows):
        for idx, it in enumerate(items):
            c, j = idx % N_CORES, idx // N_CORES
            bi = it[0]
            sw = _subw(rows)
            for m in range(nsub):
                s0, nv = it[1 + 2 * m], it[2 + 2 * m]
                fill_sub(kv_arrs[si][c, j, :, m * sw : (m + 1) * sw], rows, bi, s0, nv)
            gg = g_base[si] + j
            tq_in[c, :, gg * TQW : (gg + 1) * TQW] = tq_pack[bi]
            slot_map[c].append((gg, bi))

    pi = 0
    if pairs:
        place(pairs, sec_of[(2, False)], 2, 128)
    if singles8:
        place(singles8, sec_of[(1, False)], 1, RSINGLE)
    if singles16:
        place(singles16, sec_of[(1, True)], 1, RSINGLE)

    if plan not in _module_cache:
        _module_cache[plan] = _build_module(plan)
    nc = _module_cache[plan]

    from concourse.bass_utils import run_bass_kernel_spmd

    in_maps = []
    for c in range(N_CORES):
        m = {"tqin": tq_in[c]}
        for si in kv_arrs:
            m[f"kv{si}"] = kv_arrs[si][c]
        in_maps.append(m)
    trace = os.environ.get("BASS_KERNEL_TRACE") == "1"
    kwargs = {}
    if trace:
        kwargs = dict(trace=True, trace_cores=list(range(N_CORES)))
    res = run_bass_kernel_spmd(nc, in_maps, core_ids=list(range(N_CORES)), **kwargs)
    if trace and res.exec_time_ns is not None:
        print(f"HW exec time: {res.exec_time_ns} ns")
        print(f"HW exec time mean: {res.mean_exec_time_ns} ns")

    num = np.zeros((B, CA, H), np.float64)
    den = np.zeros((B, CA), np.float64)
    for c in range(N_CORES):
        part = res.results[c]["outp"]  # [npair_o, 128, 2, NQ] f16
        for gg, bi in slot_map[c]:
            o = part[gg // 2, :, gg % 2].astype(np.float64)  # [128, NQ]
            num[bi] += (
                o[:, : HSUB * CA].reshape(128, HSUB, CA).transpose(2, 1, 0)
            ).reshape(CA, H)
            den[bi] += o[:CA, HSUB * CA]
    out = (num / (VAL_SCALE * den[:, :, None])).astype(np.float32)
    return out


# revision 28
# speedup vs baseline: 1.1431x; 1.1431x over previous
"""Trainium2 Bass kernel for ragged-sequence attention (fp8-e3m4 stream).

Per batch b:
    tq     = tanh(query[b] @ W + bias)                      [CA, H]
    scores = key[b] @ tq.T                                  [S, CA]
    alpha  = exp(scores) * (s < seq_len[b])                 [S, CA]
    out[b] = (alpha.T @ value[b]) / alpha.sum(axis=0)[:,None]

Strategy (DMA-byte bound; the cost model treats all 16 DMA engines as one
exclusive 360 GB/s resource, so exec time ~ total bytes streamed):
  - Raggedness: independent sub-chunks of each valid prefix (<=128 rows);
    numerator/denominator are additive over s.
  - key/value stream as fp8 e3m4 (4 mantissa bits) for batches with
    L > 256; the error of the normalized attention average shrinks like
    1/sqrt(L_eff), so only short batches need f16. key is pre-scaled x32
    and value x2 to sit in e3m4's narrow range; the exp activation
    applies scale=1/32, bias=-2 (num/den ratio is bias-invariant), and
    the host divides the numerator's x2 back out.
  - Same-batch sub pairs share one tq copy and psum-accumulate into one
    output block (tq + out bytes halved). Leftover singles go in 96-row
    slots (their valid tails are short). Three per-core sections with an
    identical NEFF on every core: e3-pairs[128], e3-singles[96],
    f16-singles[96].
  - No mask / no transpose: invalid rows have zeroed value+ones-col so
    they contribute nothing; scores^T comes straight off the PE with s
    on partitions (kt stationary), and the value matmul runs transposed
    (value chunks stationary) -> out^T w-chunks [128, CA] + den [CA,1].
  - All tq's in one upfront DMA; outputs in f16, DMA'd two groups at a
    time so descriptors are >=512B (below that the model halves DMA
    bandwidth).
"""

import os
import sys

import numpy as np

for _p in ("/opt/trn_rl_repo", "/root/.axon_site/_ro/trn_rl_repo"):
    if os.path.isdir(_p) and _p not in sys.path:
        sys.path.append(_p)

N_CORES = 8
H = 768
HSUB = H // 128  # 6
CA = 32
VW = H + 1       # value cols + ones col
TQW = HSUB * CA  # 192
NQ = HSUB * CA + 1  # out cols per group: 6 w-chunks x 32 + den col

KEY_SCALE = 32.0   # key pre-scaled on host; exp() applies 1/32
VAL_SCALE = 2.0    # value pre-scaled into e3m4 range; host divides out
TQ8_SCALE = 4.0    # e3m4 tq pre-scale (exp() divides it back out)
TQ8_MIN_L = 512    # pair-groups of batches this long stream tq in e3m4
F16_MAX_L = 256    # batches with L <= this stream in f16 (short batches
                   # have too few terms for fp8 noise to average out)
RSINGLE = 96       # row granularity of single-sub slots

MODE = os.environ.get("BASS_ATTN_MODE", "auto")  # auto | f16

_module_cache = {}


def _subw(rows):
    return HSUB * rows + VW


def _build_module(plan, n8):
    """plan: sections (count, nsub, rows, is_f16); first n8 groups use
    e3m4 tq (pre-scaled x TQ8_SCALE), the rest f16 tq."""
    import concourse.mybir as mybir
    import concourse.tile as tile
    from concourse import bacc

    f32 = mybir.dt.float32
    f16 = mybir.dt.float16
    e3 = f16 if os.environ.get("BASS_E3_AS_F16") == "1" else mybir.dt.float8e3
    AF = mybir.ActivationFunctionType

    ngrp = sum(c for c, _, _, _ in plan)
    npair = (ngrp + 3) // 4

    nc = bacc.Bacc(None, target_bir_lowering=False, enable_asserts=False)
    tq8_d = (
        nc.dram_tensor("tqin8", [128, n8 * TQW], e3, kind="ExternalInput")
        if n8
        else None
    )
    tq16_d = nc.dram_tensor(
        "tqin", [128, (ngrp - n8) * TQW], f16, kind="ExternalInput"
    )
    kv_ds = []
    for si, (count, nsub, rows, is_f16) in enumerate(plan):
        kv_ds.append(
            nc.dram_tensor(
                f"kv{si}",
                [count, 128, nsub * _subw(rows)],
                f16 if is_f16 else e3,
                kind="ExternalInput",
            )
        )
    out_d = nc.dram_tensor("outp", [npair, 128, 4, NQ], f16, kind="ExternalOutput")

    with tile.TileContext(nc) as tc:
        with (
            tc.tile_pool(name="tqp", bufs=1) as tqp,
            tc.tile_pool(name="kvp", bufs=ngrp) as kvp,
            tc.tile_pool(name="alp", bufs=4) as alp,
            tc.tile_pool(name="obp", bufs=3) as obp,
            tc.tile_pool(name="ps_s", bufs=3, space="PSUM") as ps_s_pool,
            tc.tile_pool(name="ps_o", bufs=4, space="PSUM") as ps_o_pool,
        ):
            if n8:
                tq8_t = tqp.tile([128, n8 * TQW], e3, tag="tq8", name="tq8")
                nc.sync.dma_start(out=tq8_t, in_=tq8_d[:, :])
            tq16_t = tqp.tile(
                [128, (ngrp - n8) * TQW], f16, tag="tq16", name="tq16"
            )
            nc.sync.dma_start(out=tq16_t, in_=tq16_d[:, :])

            # software-pipelined emission: group g's scores+exp issue before
            # group g-1's value phase, so the in-order PE stream never
            # stalls waiting for the exp of the group it just scored.
            groups = []
            for si, (count, nsub, rows, is_f16) in enumerate(plan):
                for j in range(count):
                    groups.append((si, j, nsub, rows, is_f16))

            state = {}  # g -> (kv_t, al_t, nsub, rows)
            ob_box = [None]
            pend_box = [0]  # first un-flushed slot within the current ob

            n_before = next(
                (i for i, s in enumerate(groups) if s[2] == 2), len(groups)
            )

            def stage_front(g):
                si, j, nsub, rows, is_f16 = groups[g]
                kvdt = f16 if is_f16 else e3
                sw = _subw(rows)
                kv_t = kvp.tile([128, nsub * sw], kvdt, tag=f"kv{si}")
                if g >= len(groups) - 2:
                    # tail groups: land the first sub (or kt half) first so
                    # the exp->value->out chain starts earlier
                    kb = sw if nsub == 2 else HSUB * rows
                    nc.sync.dma_start(
                        out=kv_t[:, :kb], in_=kv_ds[si][j][:, :kb]
                    )
                    nc.sync.dma_start(
                        out=kv_t[:, kb:], in_=kv_ds[si][j][:, kb:]
                    )
                else:
                    nc.sync.dma_start(out=kv_t, in_=kv_ds[si][j])

                # scores^T [rows(s), CA] per sub, contracted over h in
                # 6 chunks of 128; kt stationary puts s on partitions.
                ja = g - n_before
                use8 = 0 <= ja < n8
                tq_t = tq8_t if use8 else tq16_t
                gq = ja if use8 else (g - n8 if g >= n_before + n8 else g)
                ps_s = ps_s_pool.tile([128, nsub * CA], f32, tag="ps_s")
                for m in range(nsub):
                    for ho in range(HSUB):
                        nc.tensor.matmul(
                            ps_s[:rows, m * CA : (m + 1) * CA],
                            lhsT=kv_t[
                                :, m * sw + ho * rows : m * sw + (ho + 1) * rows
                            ],
                            rhs=tq_t[
                                :, gq * TQW + ho * CA : gq * TQW + (ho + 1) * CA
                            ],
                            start=(ho == 0),
                            stop=(ho == HSUB - 1),
                        )

                al_t = alp.tile([128, nsub * CA], f16, tag="al")
                nc.scalar.activation(
                    out=al_t[:rows],
                    in_=ps_s[:rows],
                    func=AF.Exp,
                    scale=1.0
                    / (KEY_SCALE * (TQ8_SCALE if 0 <= g - n_before < n8 else 1.0)),
                )
                state[g] = (kv_t, al_t, nsub, rows)

            def stage_back(g):
                kv_t, al_t, nsub, rows = state.pop(g)
                sw = _subw(rows)
                # out^T accumulated over the group's subs: 6 w-chunks of
                # [128(w), CA] (value chunk stationary) + den at col 192
                # of the same psum tile (one PSUM->SBUF copy per group).
                ps_o = ps_o_pool.tile([128, NQ], f32, tag="ps_o")
                for q in range(HSUB):
                    for m in range(nsub):
                        voff = m * sw + HSUB * rows
                        nc.tensor.matmul(
                            ps_o[:, q * CA : (q + 1) * CA],
                            lhsT=kv_t[:rows, voff + q * 128 : voff + (q + 1) * 128],
                            rhs=al_t[:rows, m * CA : (m + 1) * CA],
                            start=(m == 0),
                            stop=(m == nsub - 1),
                        )
                for m in range(nsub):
                    voff = m * sw + HSUB * rows
                    nc.tensor.matmul(
                        ps_o[:CA, HSUB * CA : NQ],
                        lhsT=al_t[:rows, m * CA : (m + 1) * CA],
                        rhs=kv_t[:rows, voff + H : voff + H + 1],
                        start=(m == 0),
                        stop=(m == nsub - 1),
                    )

                if g % 4 == 0:
                    ob_box[0] = obp.tile([128, 4, NQ], f16, tag="ob", name="ob")
                ob = ob_box[0]
                sl_ = g % 4
                nc.vector.tensor_copy(out=ob[:, sl_, :], in_=ps_o)
                # out DMAs ride the idle Pool/SWDGE ring, flushed every two
                # groups (772B descriptors, same per-group DMA cost as 4):
                # on SP they'd head-of-line-block input issue, on ACT they
                # stall the next group's exp (seen in the trace). The last
                # two groups flush individually via ACT's HWDGE (632ns gen,
                # no serial Pool queue; exps are all done so nothing stalls).
                if sl_ == 3 or g == ngrp - 1:
                    lo = pend_box[0]
                    eng = nc.scalar if g == ngrp - 1 else nc.gpsimd
                    eng.dma_start(
                        out=out_d[g // 4, :, lo : sl_ + 1],
                        in_=ob[:, lo : sl_ + 1],
                    )
                    pend_box[0] = (sl_ + 1) % 4

            for g in range(len(groups)):
                stage_front(g)
                if g > 0:
                    stage_back(g - 1)
            stage_back(len(groups) - 1)

    nc.compile()
    return nc


def kernel(key, value, query, seq_len, W, b):
    import ml_dtypes

    key = np.ascontiguousarray(np.asarray(key, dtype=np.float32))
    value = np.ascontiguousarray(np.asarray(value, dtype=np.float32))
    query = np.asarray(query, dtype=np.float32)
    W = np.asarray(W, dtype=np.float32)
    bias = np.asarray(b, dtype=np.float32)
    sl = np.asarray(seq_len).astype(np.int64)

    B, S, H_ = key.shape
    assert H_ == H
    e3np = (
        np.float16
        if os.environ.get("BASS_E3_AS_F16") == "1"
        else ml_dtypes.float8_e3m4
    )

    # host: tiny projection  tq[b] = tanh(query[b] @ W + bias)  [B, CA, H]
    tq = np.tanh(query.reshape(B * query.shape[1], -1) @ W + bias)
    tq = tq.reshape(B, query.shape[1], H).astype(np.float32)

    def _tq_layout(bi):
        return np.ascontiguousarray(
            tq[bi].T.reshape(HSUB, 128, CA).transpose(1, 0, 2).reshape(128, TQW)
        )

    tq_pack16 = {bi: _tq_layout(bi).astype(np.float16) for bi in range(B)}
    tq_pack8 = {
        bi: (_tq_layout(bi) * TQ8_SCALE).astype(e3np) for bi in range(B)
    }

    # work items per precision class
    pairs = []      # (bi, s0_a, nv_a, s0_b, nv_b)  e3, 128-row subs
    singles96 = []  # (bi, s0, nval)  e3 odd tails, nval small
    singles16 = []  # (bi, s0, nval)  f16 short batches in 96-row chunks
    for bi in range(B):
        L = int(max(1, min(sl[bi], S)))
        if MODE == "f16" or L <= F16_MAX_L:
            for s0 in range(0, L, RSINGLE):
                singles16.append((bi, s0, min(RSINGLE, L - s0)))
            continue
        subs = [(s0, min(128, L - s0)) for s0 in range(0, L, 128)]
        for i in range(0, len(subs) - 1, 2):
            pairs.append((bi, *subs[i], *subs[i + 1]))
        if len(subs) % 2:
            s0, nv = subs[-1]
            if nv <= RSINGLE:
                singles96.append((bi, s0, nv))
            else:  # split the tail into two small singles
                h1 = (nv + 1) // 2
                singles96.append((bi, s0, h1))
                singles96.append((bi, s0 + h1, nv - h1))

    # Pair slots use floor(pairs/8) per core: two 1537-col single slots
    # cost exactly one 3074-col pair slot, but the floor avoids dummy
    # pair slots (ceil left 5 of 64 streaming zeros). Leftover pairs
    # split into full-height singles.
    nA = len(pairs) // N_CORES
    elig = [p for p in pairs if int(sl[p[0]]) > TQ8_MIN_L]
    rest = [p for p in pairs if int(sl[p[0]]) <= TQ8_MIN_L]
    n8 = min(len(elig) // N_CORES, nA)
    ordered = elig + rest
    core_pairs = [[] for _ in range(N_CORES)]
    for i, p in enumerate(ordered[: n8 * N_CORES]):
        core_pairs[i % N_CORES].append(p)
    mid = ordered[n8 * N_CORES : nA * N_CORES]
    for i, p in enumerate(mid):
        core_pairs[i % N_CORES].append(p)
    singles128 = []  # split leftover pairs into full-height singles
    for bi, s0a, nva, s0b, nvb in ordered[nA * N_CORES :]:
        singles128.append((bi, s0a, nva))
        singles128.append((bi, s0b, nvb))

    r96 = 32 * max(
        [1] + [-(-nv // 32) for _, _, nv in singles96 + singles16]
    )
    nB128 = -(-len(singles128) // N_CORES) if singles128 else 0
    nB96 = -(-len(singles96) // N_CORES) if singles96 else 0
    nB16 = -(-len(singles16) // N_CORES) if singles16 else 0
    # big singles go first so the pair stream saturates DMA; the very
    # last group is a small 96-row single, minimizing the trailing
    # serial exp->value->copy->out chain.
    plan = tuple(
        s
        for s in (
            (nB16, 1, r96, True),
            (nB128, 1, 128, False),
            (nA, 2, 128, False),
            (nB96, 1, r96, False),
        )
        if s[0]
    )
    ngrp = sum(c for c, _, _, _ in plan)
    n_before = nB16 + nB128  # singles preceding the pair section

    tq8_in = np.zeros((N_CORES, 128, n8 * TQW), e3np)
    tq16_in = np.zeros((N_CORES, 128, (ngrp - n8) * TQW), np.float16)
    kv_arrs = {}
    for si, (count, nsub, rows, is_f16) in enumerate(plan):
        dt = np.float16 if is_f16 else e3np
        kv_arrs[si] = np.zeros((N_CORES, count, 128, nsub * _subw(rows)), dt)
    slot_map = [[] for _ in range(N_CORES)]  # (global_group, bi)

    def set_tq(c, gg, bi):
        # Pair slots j < n8 (at g = n_before + j) read e3m4 tq at index
        # j; everything else reads f16 tq at a g-ordered index (the
        # module mirrors this).
        j = gg - n_before
        if 0 <= j < n8:
            tq8_in[c, :, j * TQW : (j + 1) * TQW] = tq_pack8[bi]
        else:
            go = gg - n8 if gg >= n_before + n8 else gg
            tq16_in[c, :, go * TQW : (go + 1) * TQW] = tq_pack16[bi]

    def fill_sub(arr, rows, bi, s0, nval):
        # arr: [128, _subw(rows)] view. kt cols [ho*rows + s] x KEY_SCALE;
        # vl rows [s, w] x VAL_SCALE + ones col; invalid rows stay zero.
        kc = key[bi, s0 : s0 + nval] * KEY_SCALE        # [nval, H]
        kt = kc.T.reshape(HSUB, 128, nval).transpose(1, 0, 2)
        ktp = np.zeros((128, HSUB, rows), np.float32)
        ktp[:, :, :nval] = kt
        arr[:, : HSUB * rows] = ktp.reshape(128, HSUB * rows).astype(arr.dtype)
        vb = HSUB * rows
        arr[:nval, vb : vb + H] = (value[bi, s0 : s0 + nval] * VAL_SCALE).astype(
            arr.dtype
        )
        arr[:nval, vb + H] = np.float32(1.0)

    g_base = {}
    acc = 0
    for si, (count, _, _, _) in enumerate(plan):
        g_base[si] = acc
        acc += count

    def fill_slot(si, c, j, it, nsub, rows):
        bi = it[0]
        sw = _subw(rows)
        for m in range(nsub):
            s0, nv = it[1 + 2 * m], it[2 + 2 * m]
            fill_sub(kv_arrs[si][c, j, :, m * sw : (m + 1) * sw], rows, bi, s0, nv)
        gg = g_base[si] + j
        set_tq(c, gg, bi)
        slot_map[c].append((gg, bi))

    si = 0
    for items, rows, count in (
        (singles16, r96, nB16),
        (singles128, 128, nB128),
        (None, 128, nA),
        (singles96, r96, nB96),
    ):
        if not count:
            continue
        if items is None:
            for c in range(N_CORES):
                for j, p in enumerate(core_pairs[c]):
                    fill_slot(si, c, j, p, 2, 128)
        else:
            for idx, it in enumerate(items):
                fill_slot(si, idx % N_CORES, idx // N_CORES, it, 1, rows)
        si += 1

    if (plan, n8) not in _module_cache:
        _module_cache[(plan, n8)] = _build_module(plan, n8)
    nc = _module_cache[(plan, n8)]

    from concourse.bass_utils import run_bass_kernel_spmd

    in_maps = []
    for c in range(N_CORES):
        m = {"tqin": tq16_in[c]}
        if n8:
            m["tqin8"] = tq8_in[c]
        for si in kv_arrs:
            m[f"kv{si}"] = kv_arrs[si][c]
        in_maps.append(m)
    trace = os.environ.get("BASS_KERNEL_TRACE") == "1"
    kwargs = {}
    if trace:
        kwargs = dict(trace=True, trace_cores=list(range(N_CORES)))
    res = run_bass_kernel_spmd(nc, in_maps, core_ids=list(range(N_CORES)), **kwargs)
    if trace and res.exec_time_ns is not None:
        print(f"HW exec time: {res.exec_time_ns} ns")
        print(f"HW exec time mean: {res.mean_exec_time_ns} ns")

    num = np.zeros((B, CA, H), np.float64)
    den = np.zeros((B, CA), np.float64)
    for c in range(N_CORES):
        part = res.results[c]["outp"]  # [ceil(ngrp/4), 128, 4, NQ] f16
        for gg, bi in slot_map[c]:
            o = part[gg // 4, :, gg % 4].astype(np.float64)  # [128, NQ]
            num[bi] += (
                o[:, : HSUB * CA].reshape(128, HSUB, CA).transpose(2, 1, 0)
            ).reshape(CA, H)
            den[bi] += o[:CA, HSUB * CA]
    out = (num / (VAL_SCALE * den[:, :, None])).astype(np.float32)
    return out
